# revision 17
# baseline (speedup 1.0000x reference)
"""GRU decoder kernel for Trainium2 (8 NeuronCores, SPMD data-parallel).

Problem: nn_Decoder — embedding lookup -> 256-step GRU -> vocab projection.
  B=16, T=256, H=1024, E=512, V=32000.

Sharding: data-parallel over batch (2 rows per core). All weights replicated.
No cross-core communication (collectives measured ~400us each here — any
per-step exchange is far slower than replicating the recurrence matmul).

The recurrence is weight-load bound on the PE (192 LDWEIGHTS+MATMUL pairs
per step at ~37ns/pair, dtype-independent). Optimizations over the naive
schedule:
  - time-major token packing (tok = t*2 + b): projection token tiles
    complete every 64 steps, so the vocab projection interleaves into
    steps 64..255 (one chunk emit per step) leaving only a 1/4 tail.
  - short critical gate chain: e = z*h_prev and f = 1-z are computed
    during the n-block matmuls; the exposed post-matmul chain is
    hn+bhn -> r*(.) -> +xn -> tanh -> f*n -> +e (writes bf16 hsT directly).
  - bias added via a K=1 matmul row (no gpsimd partition_broadcast).
  - logits emitted in bf16 (halves output DMA).

Per-core layouts:
  tokens: tok = t*2 + b  (b in 0..2 local batch rows, time-major)
  gate-major psum [128, 48]: col = m*2 + b where m = gate*8 + k  (gate r,z,n)
  xw SBUF [128, T*48]: col = t*48 + m*2 + b  (input-gate preactivations + bi)
  hsT SBUF [128, 8*512] bf16: col = k*512 + t*2 + b  (h AFTER step t)
"""

import sys
from contextlib import ExitStack

import numpy as np

sys.path.insert(0, "/opt/trn_rl_repo")

import concourse.bass as bass
import concourse.tile as tile
from concourse import bacc, mybir
from concourse.bass import IndirectOffsetOnAxis
from concourse.masks import make_identity

F32 = mybir.dt.float32
BF16 = mybir.dt.bfloat16
F8 = mybir.dt.float8e4
I32 = mybir.dt.int32

WH_FP8 = False  # fp8e4m3 Wh measured: no speedup (pair is NX-issue-bound,
                # not weight-load-bound) and rel err 1.8e-2 — too close to gate
ZID = True      # z-gate xw add via identity-matmul into PSUM (shorter tail)
AF = mybir.ActivationFunctionType
OP = mybir.AluOpType

B, T, H, E, V = 16, 256, 1024, 512, 32000
NCORES = 8
BPC = B // NCORES          # batch rows per core = 2
TOK = BPC * T              # tokens per core = 512
KE = 5                     # E k-tiles incl. aug row block (640 = 5*128)
KH = 8                     # H k-tiles
M3 = 24                    # gate-col blocks (3H/128)
EA = KE * 128              # 640

NV = 64                    # vocab n-chunks
VC = V // NV               # 500 cols per chunk


def build(T_steps=T, reps=1, proj=True):
    nc = bacc.Bacc("TRN2", target_bir_lowering=False, debug=False,
                   num_devices=NCORES)

    tgt = nc.dram_tensor("tgt", [TOK, 1], I32, kind="ExternalInput")
    h0T = nc.dram_tensor("h0T", [128, 2 * KH], F32, kind="ExternalInput")
    emb = nc.dram_tensor("emb", [V, E], BF16, kind="ExternalInput")
    wh = nc.dram_tensor("wh", [H, 3 * H], F8 if WH_FP8 else BF16,
                        kind="ExternalInput")
    wiA = nc.dram_tensor("wiA", [EA, 3 * H], BF16, kind="ExternalInput")
    wo = nc.dram_tensor("wo", [H, V], BF16, kind="ExternalInput")
    bo_in = nc.dram_tensor("bo_in", [1, V], BF16, kind="ExternalInput")
    bhnT = nc.dram_tensor("bhnT", [128, 2 * KH], F32, kind="ExternalInput")
    out = nc.dram_tensor("out", [TOK, V], BF16, kind="ExternalOutput")

    with tile.TileContext(nc) as tc:
        with ExitStack() as ctx:
            const = ctx.enter_context(tc.tile_pool(name="const", bufs=1))
            big = ctx.enter_context(tc.tile_pool(name="big", bufs=1))

            ident = const.tile([128, 128], BF16)
            make_identity(nc, ident[:])
            ones_row = const.tile([1, 128], BF16)
            nc.vector.memset(ones_row[:], 1.0)
            bhn_sb = const.tile([128, 2 * KH], F32)
            nc.sync.dma_start(bhn_sb[:], bhnT[:])
            bo_sb = const.tile([1, V], BF16)
            nc.sync.dma_start(bo_sb[:], bo_in[:])
            h0bf = const.tile([128, 2 * KH], BF16)
            h0f = const.tile([128, 2 * KH], F32)
            nc.sync.dma_start(h0f[:], h0T[:])
            nc.vector.tensor_copy(h0bf[:], h0f[:])
            # resident weights
            wh_sb = big.tile([128, KH * 3 * H], F8 if WH_FP8 else BF16)
            for k in range(KH):
                nc.sync.dma_start(wh_sb[:, k * 3 * H:(k + 1) * 3 * H],
                                  wh[k * 128:(k + 1) * 128, :])

            # with ZID the z-gate lives in xwz_sb (bf16); xw_sb keeps r|n only
            G = 32 if ZID else 48
            xw_sb = big.tile([128, T_steps * G], F32)
            xwz_sb = None
            if ZID:
                xwz_sb = big.tile([128, T_steps * 16], BF16, name="xwz_sb")
            hsT_sb = big.tile([128, KH * TOK], BF16)

            for rep in range(reps):
                # ---------------- Phase A: embed gather + x^T + xW ----------
                with tc.tile_pool(name="phA", bufs=1) as phA, \
                     tc.tile_pool(name="xgp", bufs=2) as xgp, \
                     tc.tile_pool(name="idxp", bufs=2) as idxp, \
                     tc.tile_pool(name="psA", bufs=2, space="PSUM") as psA, \
                     tc.tile_pool(name="psT", bufs=2, space="PSUM") as psT:
                    wi_sb = phA.tile([128, KE * 3 * H], BF16)
                    for k in range(KE):
                        nc.sync.dma_start(wi_sb[:, k * 3 * H:(k + 1) * 3 * H],
                                          wiA[k * 128:(k + 1) * 128, :])
                    xT_sb = phA.tile([128, KE * TOK], BF16)
                    # aug k-block: ones row (partition 0), zeros elsewhere
                    nc.vector.memset(xT_sb[:, 4 * TOK:5 * TOK], 0.0)
                    nc.vector.memset(xT_sb[0:1, 4 * TOK:5 * TOK], 1.0)
                    for c in range(TOK // 128):
                        idx = idxp.tile([128, 1], I32)
                        nc.sync.dma_start(idx[:], tgt[c * 128:(c + 1) * 128, :])
                        xg = xgp.tile([128, E], BF16)
                        nc.gpsimd.indirect_dma_start(
                            out=xg[:], out_offset=None, in_=emb[:],
                            in_offset=IndirectOffsetOnAxis(ap=idx[:, :1], axis=0),
                        )
                        for eb in range(4):
                            pst = psT.tile([128, 128], BF16)
                            nc.tensor.transpose(pst[:],
                                                xg[:, eb * 128:(eb + 1) * 128],
                                                ident[:])
                            nc.vector.tensor_copy(
                                xT_sb[:, eb * TOK + c * 128: eb * TOK + (c + 1) * 128],
                                pst[:])

                    # xW[tok, 3H] in gate-major column-packed layout
                    xw_view = xw_sb[:].rearrange("p (t g) -> p g t", g=G)
                    for m in range(M3):
                        ps = psA.tile([128, TOK], F32)
                        for kb in range(KE):
                            nc.tensor.matmul(
                                ps[:],
                                lhsT=wi_sb[:, kb * 3 * H + m * 128: kb * 3 * H + (m + 1) * 128],
                                rhs=xT_sb[:, kb * TOK:(kb + 1) * TOK],
                                start=(kb == 0), stop=(kb == KE - 1))
                        # psum cols are tok = t*2 + b (time-major)
                        src = ps[:].rearrange("p (t b) -> p b t", b=2)[:, :, :T_steps]
                        if ZID and 8 <= m < 16:
                            # z-gate preactivations go to a bf16 buffer (added
                            # into PSUM via identity matmul in the step loop)
                            xwz_view = xwz_sb[:].rearrange(
                                "p (t g) -> p g t", g=16)
                            nc.vector.tensor_copy(
                                xwz_view[:, (m - 8) * 2:(m - 8) * 2 + 2, :], src)
                        else:
                            gi = m * 2 if (not ZID or m < 8) else m * 2 - 16
                            nc.vector.tensor_copy(
                                xw_view[:, gi:gi + 2, :], src)

                # ---------------- Phase B: GRU recurrence -------------------
                hsT_view = hsT_sb[:].rearrange("p (k t b) -> p k t b",
                                               k=KH, b=2)
                if T_steps != T:
                    nc.vector.memset(hsT_sb[:], 0.0)

                def rhs_k(t, k):
                    if t == 0:
                        return h0bf[:, 2 * k:2 * k + 2]
                    return hsT_view[:, k, t - 1, :]

                with tc.tile_pool(name="gp", bufs=3) as gp, \
                     tc.tile_pool(name="wop", bufs=3) as wop, \
                     tc.tile_pool(name="otp", bufs=3) as otp, \
                     tc.tile_pool(name="psP", bufs=2, space="PSUM") as psPp, \
                     tc.tile_pool(name="psH", bufs=2, space="PSUM") as psH:

                    def load_wo_chunk(n):
                        wt = wop.tile([128, KH * VC], BF16, tag="wo")
                        for k in range(KH):
                            nc.sync.dma_start(
                                wt[:, k * VC:(k + 1) * VC],
                                wo[k * 128:(k + 1) * 128, n * VC:(n + 1) * VC])
                        return wt

                    def emit_proj_mms(tb, n, wt):
                        # out rows [tb*128:(tb+1)*128] = hsT token block tb
                        psP = psPp.tile([128, VC], F32)
                        for k in range(KH):
                            nc.tensor.matmul(
                                psP[:],
                                lhsT=hsT_sb[:, k * TOK + tb * 128: k * TOK + (tb + 1) * 128],
                                rhs=wt[:, k * VC:(k + 1) * VC],
                                start=(k == 0), stop=False)
                        # bias via K=1 matmul: out += ones_col.T @ bo_chunk
                        nc.tensor.matmul(
                            psP[:],
                            lhsT=ones_row[0:1, :],
                            rhs=bo_sb[0:1, n * VC:(n + 1) * VC],
                            start=False, stop=True)
                        return psP

                    def emit_proj_out(psP, tb, n):
                        ot = otp.tile([128, VC], BF16)
                        nc.vector.tensor_copy(ot[:], psP[:])
                        nc.sync.dma_start(
                            out[tb * 128:(tb + 1) * 128, n * VC:(n + 1) * VC], ot[:])

                    interleave = (T_steps == T) and proj
                    kb_view = "p (k b) -> p k b"
                    # PE m-tile order r -> n -> z: the tanh chain runs under
                    # the z-block matmuls; only add/sigmoid/mult/add of the
                    # z-update remain on the exposed critical path.
                    m_order = list(range(0, 8)) + list(range(16, 24)) + list(range(8, 16))
                    for t in range(T_steps):
                        ps = psH.tile([128, 48], F32)
                        for m in m_order:
                            zt = ZID and 8 <= m < 16
                            if ZID and m == 8:
                                # seed the z-gate PSUM group with xw_z
                                nc.tensor.matmul(
                                    ps[:, 16:32], lhsT=ident[:],
                                    rhs=xwz_sb[:, t * 16:(t + 1) * 16],
                                    start=True, stop=False,
                                    skip_group_check=True)
                            for k in range(KH):
                                nc.tensor.matmul(
                                    ps[:, m * 2:(m + 1) * 2],
                                    lhsT=wh_sb[:, k * 3 * H + m * 128: k * 3 * H + (m + 1) * 128],
                                    rhs=rhs_k(t, k),
                                    start=(k == 0 and not zt),
                                    stop=(k == KH - 1),
                                    skip_group_check=zt)

                        # interleaved projection: one (tile, chunk) emit per
                        # step from t=64 on; PE-queued after the gate matmuls
                        # so the PE stays busy during the gate DVE tail. The
                        # PSUM evacuation is deferred below the gate chain so
                        # the in-order DVE queue isn't blocked.
                        proj = None
                        if interleave and t >= 64:
                            i = t - 64
                            tb, n = i // 64, i % 64
                            if tb < 3:
                                wt_cur = load_wo_chunk(n)
                                proj = (emit_proj_mms(tb, n, wt_cur), tb, n)

                        xwt = xw_sb[:, t * G:(t + 1) * G]
                        noff = 16 if ZID else 32
                        hprev = (h0f[:].rearrange(kb_view, k=KH)
                                 if t == 0 else hsT_view[:, :, t - 1, :])
                        # r gate (ready after m-tiles 0..7)
                        ar = gp.tile([128, 16], F32)
                        nc.vector.tensor_tensor(ar[:], ps[:, 0:16], xwt[:, 0:16], OP.add)
                        r_ = gp.tile([128, 16], F32)
                        nc.scalar.activation(r_[:], ar[:], AF.Sigmoid)
                        # n candidate (after n-block m-tiles) — runs under the
                        # z-block matmuls
                        hnb = gp.tile([128, 16], F32)
                        nc.vector.tensor_tensor(hnb[:], ps[:, 32:48], bhn_sb[:], OP.add)
                        rn = gp.tile([128, 16], F32)
                        nc.vector.tensor_tensor(rn[:], r_[:], hnb[:], OP.mult)
                        an = gp.tile([128, 16], F32)
                        nc.vector.tensor_tensor(an[:], rn[:],
                                                xwt[:, noff:noff + 16], OP.add)
                        n_ = gp.tile([128, 16], F32)
                        nc.scalar.activation(n_[:], an[:], AF.Tanh)
                        d = gp.tile([128, 16], F32)
                        nc.vector.tensor_tensor(
                            d[:].rearrange(kb_view, k=KH), hprev,
                            n_[:].rearrange(kb_view, k=KH), OP.subtract)
                        # z gate (after z-block m-tiles) — exposed tail
                        z_ = gp.tile([128, 16], F32)
                        if ZID:
                            # xw_z already accumulated into PSUM
                            nc.scalar.activation(z_[:], ps[:, 16:32], AF.Sigmoid)
                        else:
                            az = gp.tile([128, 16], F32)
                            nc.vector.tensor_tensor(az[:], ps[:, 16:32],
                                                    xwt[:, 16:32], OP.add)
                            nc.scalar.activation(z_[:], az[:], AF.Sigmoid)
                        zd = gp.tile([128, 16], F32)
                        nc.vector.tensor_tensor(zd[:], z_[:], d[:], OP.mult)
                        # h = n + z*(hprev - n), written straight to bf16 hsT
                        nc.vector.tensor_tensor(
                            hsT_view[:, :, t, :],
                            n_[:].rearrange(kb_view, k=KH),
                            zd[:].rearrange(kb_view, k=KH), OP.add)

                        if proj is not None:
                            emit_proj_out(*proj)

                    # ---------- projection tail (token block 3) ------------
                    tail = (([3] if interleave else [0, 1, 2, 3]) if proj else [])
                    for n in range(NV):
                        wt = load_wo_chunk(n)
                        for tb in tail:
                            emit_proj_out(emit_proj_mms(tb, n, wt), tb, n)
                if rep != reps - 1:
                    tc.strict_bb_all_engine_barrier()

    nc.compile()
    return nc


# ---------------------------------------------------------------------------
# host side
# ---------------------------------------------------------------------------

def _pack_colmajor(vec_2d, bpc_rows):
    """[bpc, H] f32 -> [128, 2*KH] with col = 2*k + b."""
    o = np.zeros((128, 2 * KH), np.float32)
    for k in range(KH):
        for b in range(bpc_rows):
            o[:, 2 * k + b] = vec_2d[b, k * 128:(k + 1) * 128]
    return o


def make_in_maps(encoder_state, targets, embed_table, Wi, Wh, bi, bhn, Wo, bo):
    encoder_state = np.asarray(encoder_state, np.float32)
    targets = np.asarray(targets)
    embed_table = np.asarray(embed_table, np.float32)
    Wi = np.asarray(Wi, np.float32)
    Wh = np.asarray(Wh, np.float32)
    bi = np.asarray(bi, np.float32)
    bhn = np.asarray(bhn, np.float32)
    Wo = np.asarray(Wo, np.float32)
    bo = np.asarray(bo, np.float32)

    import ml_dtypes
    emb_bf = embed_table.astype(ml_dtypes.bfloat16)
    wh_bf = Wh.astype(ml_dtypes.float8_e4m3 if WH_FP8 else ml_dtypes.bfloat16)
    wiA = np.zeros((EA, 3 * H), np.float32)
    wiA[:E] = Wi
    wiA[E] = bi
    wiA_bf = wiA.astype(ml_dtypes.bfloat16)
    wo_bf = Wo.astype(ml_dtypes.bfloat16)
    bo_row = bo.reshape(1, V).astype(ml_dtypes.bfloat16)

    bhn_pack = _pack_colmajor(np.broadcast_to(bhn, (BPC, H)), BPC)

    in_maps = []
    for c in range(NCORES):
        rows = slice(c * BPC, (c + 1) * BPC)
        # time-major token order: tok = t*2 + b
        tgt = np.ascontiguousarray(
            targets[rows].T).reshape(TOK, 1).astype(np.int32)
        h0 = _pack_colmajor(encoder_state[rows], BPC)
        in_maps.append({
            "tgt": tgt,
            "h0T": h0,
            "emb": emb_bf,
            "wh": wh_bf,
            "wiA": wiA_bf,
            "wo": wo_bf,
            "bo_in": bo_row,
            "bhnT": bhn_pack,
        })
    return in_maps


_NC_CACHE = {}


def get_nc(T_steps=T, reps=1):
    key = (T_steps, reps)
    if key not in _NC_CACHE:
        _NC_CACHE[key] = build(T_steps, reps)
    return _NC_CACHE[key]


def kernel(encoder_state, targets, embed_table, Wi, Wh, bi, bhn, Wo, bo):
    from concourse.bass_utils import run_bass_kernel_spmd
    nc = get_nc()
    in_maps = make_in_maps(encoder_state, targets, embed_table, Wi, Wh, bi,
                           bhn, Wo, bo)
    res = run_bass_kernel_spmd(nc, in_maps, list(range(NCORES)))
    outs = []
    for c in range(NCORES):
        o = res.results[c]["out"].astype(np.float32)
        # rows are tok = t*2 + b -> [BPC, T, V]
        outs.append(o.reshape(T, BPC, V).transpose(1, 0, 2))
    return np.concatenate(outs, axis=0)



# revision 18
# speedup vs baseline: 2.5636x; 2.5636x over previous
"""GRU decoder kernel for Trainium2 (8 NeuronCores, SPMD data-parallel).

Problem: nn_Decoder — embedding lookup -> 256-step GRU -> vocab projection.
  B=16, T=256, H=1024, E=512, V=32000.

Sharding: data-parallel over batch (2 rows per core). All weights replicated.
No cross-core communication (collectives measured ~400us each here — any
per-step exchange is far slower than replicating the recurrence matmul).

The recurrence is weight-load bound on the PE (192 LDWEIGHTS+MATMUL pairs
per step at ~37ns/pair, dtype-independent). Optimizations over the naive
schedule:
  - time-major token packing (tok = t*2 + b): projection token tiles
    complete every 64 steps, so the vocab projection interleaves into
    steps 64..255 (one chunk emit per step) leaving only a 1/4 tail.
  - short critical gate chain: e = z*h_prev and f = 1-z are computed
    during the n-block matmuls; the exposed post-matmul chain is
    hn+bhn -> r*(.) -> +xn -> tanh -> f*n -> +e (writes bf16 hsT directly).
  - bias added via a K=1 matmul row (no gpsimd partition_broadcast).
  - logits emitted in bf16 (halves output DMA).

Per-core layouts:
  tokens: tok = t*2 + b  (b in 0..2 local batch rows, time-major)
  gate-major psum [128, 48]: col = m*2 + b where m = gate*8 + k  (gate r,z,n)
  xw SBUF [128, T*48]: col = t*48 + m*2 + b  (input-gate preactivations + bi)
  hsT SBUF [128, 8*512] bf16: col = k*512 + t*2 + b  (h AFTER step t)
"""

import sys
from contextlib import ExitStack

import numpy as np

sys.path.insert(0, "/opt/trn_rl_repo")

import concourse.bass as bass
import concourse.tile as tile
from concourse import bacc, mybir
from concourse.bass import IndirectOffsetOnAxis
from concourse.masks import make_identity

F32 = mybir.dt.float32
BF16 = mybir.dt.bfloat16
F8 = mybir.dt.float8e4
I32 = mybir.dt.int32

WH_FP8 = False  # fp8e4m3 Wh measured: no speedup (pair is NX-issue-bound,
                # not weight-load-bound) and rel err 1.8e-2 — too close to gate
ZID = False     # z-gate xw add via identity-matmul into PSUM: A/B-measured
                # neutral-to-worse (tail already hidden by proj interleave;
                # the per-step identity MM adds PE-stream work)
AF = mybir.ActivationFunctionType
OP = mybir.AluOpType

B, T, H, E, V = 16, 256, 1024, 512, 32000
NCORES = 8
BPC = B // NCORES          # batch rows per core = 2
TOK = BPC * T              # tokens per core = 512
KE = 5                     # E k-tiles incl. aug row block (640 = 5*128)
KH = 8                     # H k-tiles
M3 = 24                    # gate-col blocks (3H/128)
EA = KE * 128              # 640

NV = 64                    # vocab n-chunks
VC = V // NV               # 500 cols per chunk


def build(T_steps=T, reps=1, proj=True):
    nc = bacc.Bacc("TRN2", target_bir_lowering=False, debug=False,
                   num_devices=NCORES)

    tgt = nc.dram_tensor("tgt", [TOK, 1], I32, kind="ExternalInput")
    h0T = nc.dram_tensor("h0T", [128, 2 * KH], F32, kind="ExternalInput")
    emb = nc.dram_tensor("emb", [V, E], BF16, kind="ExternalInput")
    wh = nc.dram_tensor("wh", [H, 3 * H], F8 if WH_FP8 else BF16,
                        kind="ExternalInput")
    wiA = nc.dram_tensor("wiA", [EA, 3 * H], BF16, kind="ExternalInput")
    wo = nc.dram_tensor("wo", [H, V], BF16, kind="ExternalInput")
    bo_in = nc.dram_tensor("bo_in", [1, V], BF16, kind="ExternalInput")
    bhnT = nc.dram_tensor("bhnT", [128, 2 * KH], F32, kind="ExternalInput")
    out = nc.dram_tensor("out", [TOK, V], BF16, kind="ExternalOutput")

    with tile.TileContext(nc) as tc:
        with ExitStack() as ctx:
            const = ctx.enter_context(tc.tile_pool(name="const", bufs=1))
            big = ctx.enter_context(tc.tile_pool(name="big", bufs=1))

            ident = const.tile([128, 128], BF16)
            make_identity(nc, ident[:])
            ones_row = const.tile([1, 128], BF16)
            nc.vector.memset(ones_row[:], 1.0)
            bhn_sb = const.tile([128, 2 * KH], F32)
            nc.sync.dma_start(bhn_sb[:], bhnT[:])
            bo_sb = const.tile([1, V], BF16)
            nc.sync.dma_start(bo_sb[:], bo_in[:])
            h0bf = const.tile([128, 2 * KH], BF16)
            h0f = const.tile([128, 2 * KH], F32)
            nc.sync.dma_start(h0f[:], h0T[:])
            nc.vector.tensor_copy(h0bf[:], h0f[:])
            # resident weights
            wh_sb = big.tile([128, KH * 3 * H], F8 if WH_FP8 else BF16)
            for k in range(KH):
                nc.sync.dma_start(wh_sb[:, k * 3 * H:(k + 1) * 3 * H],
                                  wh[k * 128:(k + 1) * 128, :])

            # with ZID the z-gate lives in xwz_sb (bf16); xw_sb keeps r|n only
            G = 32 if ZID else 48
            xw_sb = big.tile([128, T_steps * G], F32)
            xwz_sb = None
            if ZID:
                xwz_sb = big.tile([128, T_steps * 16], BF16, name="xwz_sb")
            hsT_sb = big.tile([128, KH * TOK], BF16)

            for rep in range(reps):
                # ---------------- Phase A: embed gather + x^T + xW ----------
                with tc.tile_pool(name="phA", bufs=1) as phA, \
                     tc.tile_pool(name="xgp", bufs=2) as xgp, \
                     tc.tile_pool(name="idxp", bufs=2) as idxp, \
                     tc.tile_pool(name="psA", bufs=2, space="PSUM") as psA, \
                     tc.tile_pool(name="psT", bufs=2, space="PSUM") as psT:
                    wi_sb = phA.tile([128, KE * 3 * H], BF16)
                    for k in range(KE):
                        nc.sync.dma_start(wi_sb[:, k * 3 * H:(k + 1) * 3 * H],
                                          wiA[k * 128:(k + 1) * 128, :])
                    xT_sb = phA.tile([128, KE * TOK], BF16)
                    # aug k-block: ones row (partition 0), zeros elsewhere
                    nc.vector.memset(xT_sb[:, 4 * TOK:5 * TOK], 0.0)
                    nc.vector.memset(xT_sb[0:1, 4 * TOK:5 * TOK], 1.0)
                    for c in range(TOK // 128):
                        idx = idxp.tile([128, 1], I32)
                        nc.sync.dma_start(idx[:], tgt[c * 128:(c + 1) * 128, :])
                        xg = xgp.tile([128, E], BF16)
                        nc.gpsimd.indirect_dma_start(
                            out=xg[:], out_offset=None, in_=emb[:],
                            in_offset=IndirectOffsetOnAxis(ap=idx[:, :1], axis=0),
                        )
                        for eb in range(4):
                            pst = psT.tile([128, 128], BF16)
                            nc.tensor.transpose(pst[:],
                                                xg[:, eb * 128:(eb + 1) * 128],
                                                ident[:])
                            nc.vector.tensor_copy(
                                xT_sb[:, eb * TOK + c * 128: eb * TOK + (c + 1) * 128],
                                pst[:])

                    # xW[tok, 3H] in gate-major column-packed layout
                    xw_view = xw_sb[:].rearrange("p (t g) -> p g t", g=G)
                    for m in range(M3):
                        ps = psA.tile([128, TOK], F32)
                        for kb in range(KE):
                            nc.tensor.matmul(
                                ps[:],
                                lhsT=wi_sb[:, kb * 3 * H + m * 128: kb * 3 * H + (m + 1) * 128],
                                rhs=xT_sb[:, kb * TOK:(kb + 1) * TOK],
                                start=(kb == 0), stop=(kb == KE - 1))
                        # psum cols are tok = t*2 + b (time-major)
                        src = ps[:].rearrange("p (t b) -> p b t", b=2)[:, :, :T_steps]
                        if ZID and 8 <= m < 16:
                            # z-gate preactivations go to a bf16 buffer (added
                            # into PSUM via identity matmul in the step loop)
                            xwz_view = xwz_sb[:].rearrange(
                                "p (t g) -> p g t", g=16)
                            nc.vector.tensor_copy(
                                xwz_view[:, (m - 8) * 2:(m - 8) * 2 + 2, :], src)
                        else:
                            gi = m * 2 if (not ZID or m < 8) else m * 2 - 16
                            nc.vector.tensor_copy(
                                xw_view[:, gi:gi + 2, :], src)

                # ---------------- Phase B: GRU recurrence -------------------
                hsT_view = hsT_sb[:].rearrange("p (k t b) -> p k t b",
                                               k=KH, b=2)
                if T_steps != T:
                    nc.vector.memset(hsT_sb[:], 0.0)

                def rhs_k(t, k):
                    if t == 0:
                        return h0bf[:, 2 * k:2 * k + 2]
                    return hsT_view[:, k, t - 1, :]

                with tc.tile_pool(name="gp", bufs=3) as gp, \
                     tc.tile_pool(name="wop", bufs=3) as wop, \
                     tc.tile_pool(name="otp", bufs=3) as otp, \
                     tc.tile_pool(name="psP", bufs=2, space="PSUM") as psPp, \
                     tc.tile_pool(name="psH", bufs=2, space="PSUM") as psH:

                    def load_wo_chunk(n):
                        wt = wop.tile([128, KH * VC], BF16, tag="wo")
                        for k in range(KH):
                            nc.sync.dma_start(
                                wt[:, k * VC:(k + 1) * VC],
                                wo[k * 128:(k + 1) * 128, n * VC:(n + 1) * VC])
                        return wt

                    def emit_proj_mms(tb, n, wt):
                        # out rows [tb*128:(tb+1)*128] = hsT token block tb
                        psP = psPp.tile([128, VC], F32)
                        for k in range(KH):
                            nc.tensor.matmul(
                                psP[:],
                                lhsT=hsT_sb[:, k * TOK + tb * 128: k * TOK + (tb + 1) * 128],
                                rhs=wt[:, k * VC:(k + 1) * VC],
                                start=(k == 0), stop=False)
                        # bias via K=1 matmul: out += ones_col.T @ bo_chunk
                        nc.tensor.matmul(
                            psP[:],
                            lhsT=ones_row[0:1, :],
                            rhs=bo_sb[0:1, n * VC:(n + 1) * VC],
                            start=False, stop=True)
                        return psP

                    def emit_proj_out(psP, tb, n):
                        ot = otp.tile([128, VC], BF16)
                        nc.vector.tensor_copy(ot[:], psP[:])
                        nc.sync.dma_start(
                            out[tb * 128:(tb + 1) * 128, n * VC:(n + 1) * VC], ot[:])

                    interleave = (T_steps == T) and proj
                    kb_view = "p (k b) -> p k b"
                    # PE m-tile order r -> n -> z: the tanh chain runs under
                    # the z-block matmuls; only add/sigmoid/mult/add of the
                    # z-update remain on the exposed critical path.
                    m_order = list(range(0, 8)) + list(range(16, 24)) + list(range(8, 16))
                    for t in range(T_steps):
                        ps = psH.tile([128, 48], F32)
                        for m in m_order:
                            zt = ZID and 8 <= m < 16
                            if ZID and m == 8:
                                # seed the z-gate PSUM group with xw_z
                                nc.tensor.matmul(
                                    ps[:, 16:32], lhsT=ident[:],
                                    rhs=xwz_sb[:, t * 16:(t + 1) * 16],
                                    start=True, stop=False,
                                    skip_group_check=True)
                            for k in range(KH):
                                nc.tensor.matmul(
                                    ps[:, m * 2:(m + 1) * 2],
                                    lhsT=wh_sb[:, k * 3 * H + m * 128: k * 3 * H + (m + 1) * 128],
                                    rhs=rhs_k(t, k),
                                    start=(k == 0 and not zt),
                                    stop=(k == KH - 1),
                                    skip_group_check=zt)

                        # interleaved projection: one (tile, chunk) emit per
                        # step from t=64 on; PE-queued after the gate matmuls
                        # so the PE stays busy during the gate DVE tail. The
                        # PSUM evacuation is deferred below the gate chain so
                        # the in-order DVE queue isn't blocked.
                        proj = None
                        if interleave and t >= 64:
                            i = t - 64
                            tb, n = i // 64, i % 64
                            if tb < 3:
                                wt_cur = load_wo_chunk(n)
                                proj = (emit_proj_mms(tb, n, wt_cur), tb, n)

                        xwt = xw_sb[:, t * G:(t + 1) * G]
                        noff = 16 if ZID else 32
                        hprev = (h0f[:].rearrange(kb_view, k=KH)
                                 if t == 0 else hsT_view[:, :, t - 1, :])
                        # r gate (ready after m-tiles 0..7)
                        ar = gp.tile([128, 16], F32)
                        nc.vector.tensor_tensor(ar[:], ps[:, 0:16], xwt[:, 0:16], OP.add)
                        r_ = gp.tile([128, 16], F32)
                        nc.scalar.activation(r_[:], ar[:], AF.Sigmoid)
                        # n candidate (after n-block m-tiles) — runs under the
                        # z-block matmuls
                        hnb = gp.tile([128, 16], F32)
                        nc.vector.tensor_tensor(hnb[:], ps[:, 32:48], bhn_sb[:], OP.add)
                        rn = gp.tile([128, 16], F32)
                        nc.vector.tensor_tensor(rn[:], r_[:], hnb[:], OP.mult)
                        an = gp.tile([128, 16], F32)
                        nc.vector.tensor_tensor(an[:], rn[:],
                                                xwt[:, noff:noff + 16], OP.add)
                        n_ = gp.tile([128, 16], F32)
                        nc.scalar.activation(n_[:], an[:], AF.Tanh)
                        d = gp.tile([128, 16], F32)
                        nc.vector.tensor_tensor(
                            d[:].rearrange(kb_view, k=KH), hprev,
                            n_[:].rearrange(kb_view, k=KH), OP.subtract)
                        # z gate (after z-block m-tiles) — exposed tail
                        z_ = gp.tile([128, 16], F32)
                        if ZID:
                            # xw_z already accumulated into PSUM
                            nc.scalar.activation(z_[:], ps[:, 16:32], AF.Sigmoid)
                        else:
                            az = gp.tile([128, 16], F32)
                            nc.vector.tensor_tensor(az[:], ps[:, 16:32],
                                                    xwt[:, 16:32], OP.add)
                            nc.scalar.activation(z_[:], az[:], AF.Sigmoid)
                        zd = gp.tile([128, 16], F32)
                        nc.vector.tensor_tensor(zd[:], z_[:], d[:], OP.mult)
                        # h = n + z*(hprev - n), written straight to bf16 hsT
                        nc.vector.tensor_tensor(
                            hsT_view[:, :, t, :],
                            n_[:].rearrange(kb_view, k=KH),
                            zd[:].rearrange(kb_view, k=KH), OP.add)

                        if proj is not None:
                            emit_proj_out(*proj)

                    # ---------- projection tail (token block 3) ------------
                    tail = (([3] if interleave else [0, 1, 2, 3]) if proj else [])
                    for n in range(NV):
                        wt = load_wo_chunk(n)
                        for tb in tail:
                            emit_proj_out(emit_proj_mms(tb, n, wt), tb, n)
                if rep != reps - 1:
                    tc.strict_bb_all_engine_barrier()

    nc.compile()
    return nc


# ---------------------------------------------------------------------------
# host side
# ---------------------------------------------------------------------------

def _pack_colmajor(vec_2d, bpc_rows):
    """[bpc, H] f32 -> [128, 2*KH] with col = 2*k + b."""
    o = np.zeros((128, 2 * KH), np.float32)
    for k in range(KH):
        for b in range(bpc_rows):
            o[:, 2 * k + b] = vec_2d[b, k * 128:(k + 1) * 128]
    return o


def make_in_maps(encoder_state, targets, embed_table, Wi, Wh, bi, bhn, Wo, bo):
    encoder_state = np.asarray(encoder_state, np.float32)
    targets = np.asarray(targets)
    embed_table = np.asarray(embed_table, np.float32)
    Wi = np.asarray(Wi, np.float32)
    Wh = np.asarray(Wh, np.float32)
    bi = np.asarray(bi, np.float32)
    bhn = np.asarray(bhn, np.float32)
    Wo = np.asarray(Wo, np.float32)
    bo = np.asarray(bo, np.float32)

    import ml_dtypes
    emb_bf = embed_table.astype(ml_dtypes.bfloat16)
    wh_bf = Wh.astype(ml_dtypes.float8_e4m3 if WH_FP8 else ml_dtypes.bfloat16)
    wiA = np.zeros((EA, 3 * H), np.float32)
    wiA[:E] = Wi
    wiA[E] = bi
    wiA_bf = wiA.astype(ml_dtypes.bfloat16)
    wo_bf = Wo.astype(ml_dtypes.bfloat16)
    bo_row = bo.reshape(1, V).astype(ml_dtypes.bfloat16)

    bhn_pack = _pack_colmajor(np.broadcast_to(bhn, (BPC, H)), BPC)

    in_maps = []
    for c in range(NCORES):
        rows = slice(c * BPC, (c + 1) * BPC)
        # time-major token order: tok = t*2 + b
        tgt = np.ascontiguousarray(
            targets[rows].T).reshape(TOK, 1).astype(np.int32)
        h0 = _pack_colmajor(encoder_state[rows], BPC)
        in_maps.append({
            "tgt": tgt,
            "h0T": h0,
            "emb": emb_bf,
            "wh": wh_bf,
            "wiA": wiA_bf,
            "wo": wo_bf,
            "bo_in": bo_row,
            "bhnT": bhn_pack,
        })
    return in_maps


_NC_CACHE = {}


def get_nc(T_steps=T, reps=1):
    key = (T_steps, reps)
    if key not in _NC_CACHE:
        _NC_CACHE[key] = build(T_steps, reps)
    return _NC_CACHE[key]


def kernel(encoder_state, targets, embed_table, Wi, Wh, bi, bhn, Wo, bo):
    from concourse.bass_utils import run_bass_kernel_spmd
    nc = get_nc()
    in_maps = make_in_maps(encoder_state, targets, embed_table, Wi, Wh, bi,
                           bhn, Wo, bo)
    res = run_bass_kernel_spmd(nc, in_maps, list(range(NCORES)))
    outs = []
    for c in range(NCORES):
        o = res.results[c]["out"].astype(np.float32)
        # rows are tok = t*2 + b -> [BPC, T, V]
        outs.append(o.reshape(T, BPC, V).transpose(1, 0, 2))
    return np.concatenate(outs, axis=0)



# revision 20
# speedup vs baseline: 3.3926x; 1.3234x over previous
"""GRU decoder kernel for Trainium2 (8 NeuronCores, SPMD data-parallel).

Problem: nn_Decoder — embedding lookup -> 256-step GRU -> vocab projection.
  B=16, T=256, H=1024, E=512, V=32000.

Sharding: data-parallel over batch (2 rows per core). All weights replicated.
No cross-core communication (collectives measured ~400us each here — any
per-step exchange is far slower than replicating the recurrence matmul).

The recurrence is weight-load bound on the PE (192 LDWEIGHTS+MATMUL pairs
per step at ~37ns/pair, dtype-independent). Optimizations over the naive
schedule:
  - time-major token packing (tok = t*2 + b): projection token tiles
    complete every 64 steps, so the vocab projection interleaves into
    steps 64..255 (one chunk emit per step) leaving only a 1/4 tail.
  - short critical gate chain: e = z*h_prev and f = 1-z are computed
    during the n-block matmuls; the exposed post-matmul chain is
    hn+bhn -> r*(.) -> +xn -> tanh -> f*n -> +e (writes bf16 hsT directly).
  - bias added via a K=1 matmul row (no gpsimd partition_broadcast).
  - logits emitted in bf16 (halves output DMA).

Per-core layouts:
  tokens: tok = t*2 + b  (b in 0..2 local batch rows, time-major)
  gate-major psum [128, 48]: col = m*2 + b where m = gate*8 + k  (gate r,z,n)
  xw SBUF [128, T*48]: col = t*48 + m*2 + b  (input-gate preactivations + bi)
  hsT SBUF [128, 8*512] bf16: col = k*512 + t*2 + b  (h AFTER step t)
"""

import sys
from contextlib import ExitStack

import numpy as np

sys.path.insert(0, "/opt/trn_rl_repo")

import concourse.bass as bass
import concourse.tile as tile
from concourse import bacc, mybir
from concourse.bass import IndirectOffsetOnAxis
from concourse.masks import make_identity

F32 = mybir.dt.float32
BF16 = mybir.dt.bfloat16
F8 = mybir.dt.float8e4
I32 = mybir.dt.int32

WH_FP8 = False  # fp8e4m3 Wh measured: no speedup (pair is NX-issue-bound,
                # not weight-load-bound) and rel err 1.8e-2 — too close to gate
ZID = False     # z-gate xw add via identity-matmul into PSUM: A/B-measured
                # neutral-to-worse (tail already hidden by proj interleave;
                # the per-step identity MM adds PE-stream work)
AF = mybir.ActivationFunctionType
OP = mybir.AluOpType

B, T, H, E, V = 16, 256, 1024, 512, 32000
NCORES = 8
BPC = B // NCORES          # batch rows per core = 2
TOK = BPC * T              # tokens per core = 512
KE = 5                     # E k-tiles incl. aug row block (640 = 5*128)
KH = 8                     # H k-tiles
M3 = 24                    # gate-col blocks (3H/128)
EA = KE * 128              # 640

NV = 64                    # vocab n-chunks
VC = V // NV               # 500 cols per chunk


def build(T_steps=T, reps=1, proj=True):
    nc = bacc.Bacc("TRN2", target_bir_lowering=False, debug=False,
                   num_devices=NCORES)

    tgt = nc.dram_tensor("tgt", [TOK, 1], I32, kind="ExternalInput")
    h0T = nc.dram_tensor("h0T", [128, 2 * KH], F32, kind="ExternalInput")
    emb = nc.dram_tensor("emb", [V, E], BF16, kind="ExternalInput")
    wh = nc.dram_tensor("wh", [H, 3 * H], F8 if WH_FP8 else BF16,
                        kind="ExternalInput")
    wiA = nc.dram_tensor("wiA", [EA, 3 * H], BF16, kind="ExternalInput")
    wo = nc.dram_tensor("wo", [H, V], BF16, kind="ExternalInput")
    bo_in = nc.dram_tensor("bo_in", [1, V], BF16, kind="ExternalInput")
    bhnT = nc.dram_tensor("bhnT", [128, 2 * KH], F32, kind="ExternalInput")
    out = nc.dram_tensor("out", [TOK, V], BF16, kind="ExternalOutput")

    with tile.TileContext(nc) as tc:
        with ExitStack() as ctx:
            const = ctx.enter_context(tc.tile_pool(name="const", bufs=1))
            big = ctx.enter_context(tc.tile_pool(name="big", bufs=1))

            ident = const.tile([128, 128], BF16)
            make_identity(nc, ident[:])
            ones_row = const.tile([1, 128], BF16)
            nc.vector.memset(ones_row[:], 1.0)
            bhn_sb = const.tile([128, 2 * KH], F32)
            nc.sync.dma_start(bhn_sb[:], bhnT[:])
            bo_sb = const.tile([1, V], BF16)
            nc.sync.dma_start(bo_sb[:], bo_in[:])
            h0bf = const.tile([128, 2 * KH], BF16)
            h0f = const.tile([128, 2 * KH], F32)
            nc.sync.dma_start(h0f[:], h0T[:])
            nc.vector.tensor_copy(h0bf[:], h0f[:])
            # resident weights
            wh_sb = big.tile([128, KH * 3 * H], F8 if WH_FP8 else BF16)
            for k in range(KH):
                nc.sync.dma_start(wh_sb[:, k * 3 * H:(k + 1) * 3 * H],
                                  wh[k * 128:(k + 1) * 128, :])

            # with ZID the z-gate lives in xwz_sb (bf16); xw_sb keeps r|n only
            G = 32 if ZID else 48
            xw_sb = big.tile([128, T_steps * G], F32)
            xwz_sb = None
            if ZID:
                xwz_sb = big.tile([128, T_steps * 16], BF16, name="xwz_sb")
            hsT_sb = big.tile([128, KH * TOK], BF16)

            for rep in range(reps):
                # ---------------- Phase A: embed gather + x^T + xW ----------
                with tc.tile_pool(name="phA", bufs=1) as phA, \
                     tc.tile_pool(name="xgp", bufs=2) as xgp, \
                     tc.tile_pool(name="idxp", bufs=2) as idxp, \
                     tc.tile_pool(name="psA", bufs=2, space="PSUM") as psA, \
                     tc.tile_pool(name="psT", bufs=2, space="PSUM") as psT:
                    wi_sb = phA.tile([128, KE * 3 * H], BF16)
                    for k in range(KE):
                        nc.sync.dma_start(wi_sb[:, k * 3 * H:(k + 1) * 3 * H],
                                          wiA[k * 128:(k + 1) * 128, :])
                    xT_sb = phA.tile([128, KE * TOK], BF16)
                    # aug k-block: ones row (partition 0), zeros elsewhere
                    nc.vector.memset(xT_sb[:, 4 * TOK:5 * TOK], 0.0)
                    nc.vector.memset(xT_sb[0:1, 4 * TOK:5 * TOK], 1.0)
                    for c in range(TOK // 128):
                        idx = idxp.tile([128, 1], I32)
                        nc.sync.dma_start(idx[:], tgt[c * 128:(c + 1) * 128, :])
                        xg = xgp.tile([128, E], BF16)
                        nc.gpsimd.indirect_dma_start(
                            out=xg[:], out_offset=None, in_=emb[:],
                            in_offset=IndirectOffsetOnAxis(ap=idx[:, :1], axis=0),
                        )
                        for eb in range(4):
                            pst = psT.tile([128, 128], BF16)
                            nc.tensor.transpose(pst[:],
                                                xg[:, eb * 128:(eb + 1) * 128],
                                                ident[:])
                            nc.vector.tensor_copy(
                                xT_sb[:, eb * TOK + c * 128: eb * TOK + (c + 1) * 128],
                                pst[:])

                    # xW[tok, 3H] in gate-major column-packed layout
                    xw_view = xw_sb[:].rearrange("p (t g) -> p g t", g=G)
                    for m in range(M3):
                        ps = psA.tile([128, TOK], F32)
                        for kb in range(KE):
                            nc.tensor.matmul(
                                ps[:],
                                lhsT=wi_sb[:, kb * 3 * H + m * 128: kb * 3 * H + (m + 1) * 128],
                                rhs=xT_sb[:, kb * TOK:(kb + 1) * TOK],
                                start=(kb == 0), stop=(kb == KE - 1))
                        # psum cols are tok = t*2 + b (time-major)
                        src = ps[:].rearrange("p (t b) -> p b t", b=2)[:, :, :T_steps]
                        if ZID and 8 <= m < 16:
                            # z-gate preactivations go to a bf16 buffer (added
                            # into PSUM via identity matmul in the step loop)
                            xwz_view = xwz_sb[:].rearrange(
                                "p (t g) -> p g t", g=16)
                            nc.vector.tensor_copy(
                                xwz_view[:, (m - 8) * 2:(m - 8) * 2 + 2, :], src)
                        else:
                            gi = m * 2 if (not ZID or m < 8) else m * 2 - 16
                            nc.vector.tensor_copy(
                                xw_view[:, gi:gi + 2, :], src)

                # ---------------- Phase B: GRU recurrence -------------------
                hsT_view = hsT_sb[:].rearrange("p (k t b) -> p k t b",
                                               k=KH, b=2)
                if T_steps != T:
                    nc.vector.memset(hsT_sb[:], 0.0)

                def rhs_k(t, k):
                    if t == 0:
                        return h0bf[:, 2 * k:2 * k + 2]
                    return hsT_view[:, k, t - 1, :]

                with tc.tile_pool(name="gp", bufs=3) as gp, \
                     tc.tile_pool(name="wop", bufs=3) as wop, \
                     tc.tile_pool(name="otp", bufs=3) as otp, \
                     tc.tile_pool(name="psP", bufs=2, space="PSUM") as psPp, \
                     tc.tile_pool(name="psH", bufs=2, space="PSUM") as psH:

                    def load_wo_chunk(n):
                        # one 3D-AP DMA for all KH k-slices (8x fewer DGE
                        # issues; keeps the sync queue clear of wo traffic)
                        wt = wop.tile([128, KH * VC], BF16, tag="wo")
                        src = wo[:].rearrange("(k p) v -> p k v", k=KH)
                        nc.sync.dma_start(
                            wt[:].rearrange("p (k c) -> p k c", k=KH),
                            src[:, :, n * VC:(n + 1) * VC])
                        return wt

                    def emit_proj_mms(tb, n, wt):
                        # out rows [tb*128:(tb+1)*128] = hsT token block tb
                        psP = psPp.tile([128, VC], F32)
                        for k in range(KH):
                            nc.tensor.matmul(
                                psP[:],
                                lhsT=hsT_sb[:, k * TOK + tb * 128: k * TOK + (tb + 1) * 128],
                                rhs=wt[:, k * VC:(k + 1) * VC],
                                start=(k == 0), stop=False)
                        # bias via K=1 matmul: out += ones_col.T @ bo_chunk
                        nc.tensor.matmul(
                            psP[:],
                            lhsT=ones_row[0:1, :],
                            rhs=bo_sb[0:1, n * VC:(n + 1) * VC],
                            start=False, stop=True)
                        return psP

                    def emit_proj_out(psP, tb, n):
                        ot = otp.tile([128, VC], BF16)
                        nc.vector.tensor_copy(ot[:], psP[:])
                        # logits store on the ACT DGE queue: keeps the big
                        # output traffic off the sync queue that feeds wo
                        nc.scalar.dma_start(
                            out[tb * 128:(tb + 1) * 128, n * VC:(n + 1) * VC], ot[:])

                    interleave = (T_steps == T) and proj
                    kb_view = "p (k b) -> p k b"
                    # PE m-tile order r -> n -> z: the tanh chain runs under
                    # the z-block matmuls; only add/sigmoid/mult/add of the
                    # z-update remain on the exposed critical path.
                    m_order = list(range(0, 8)) + list(range(16, 24)) + list(range(8, 16))
                    for t in range(T_steps):
                        ps = psH.tile([128, 48], F32)
                        for m in m_order:
                            zt = ZID and 8 <= m < 16
                            if ZID and m == 8:
                                # seed the z-gate PSUM group with xw_z
                                nc.tensor.matmul(
                                    ps[:, 16:32], lhsT=ident[:],
                                    rhs=xwz_sb[:, t * 16:(t + 1) * 16],
                                    start=True, stop=False,
                                    skip_group_check=True)
                            for k in range(KH):
                                nc.tensor.matmul(
                                    ps[:, m * 2:(m + 1) * 2],
                                    lhsT=wh_sb[:, k * 3 * H + m * 128: k * 3 * H + (m + 1) * 128],
                                    rhs=rhs_k(t, k),
                                    start=(k == 0 and not zt),
                                    stop=(k == KH - 1),
                                    skip_group_check=zt)

                        # interleaved projection: one (tile, chunk) emit per
                        # step from t=64 on; PE-queued after the gate matmuls
                        # so the PE stays busy during the gate DVE tail. The
                        # PSUM evacuation is deferred below the gate chain so
                        # the in-order DVE queue isn't blocked.
                        proj = None
                        if interleave and t >= 64:
                            i = t - 64
                            tb, n = i // 64, i % 64
                            if tb < 3:
                                wt_cur = load_wo_chunk(n)
                                proj = (emit_proj_mms(tb, n, wt_cur), tb, n)

                        xwt = xw_sb[:, t * G:(t + 1) * G]
                        noff = 16 if ZID else 32
                        hprev = (h0f[:].rearrange(kb_view, k=KH)
                                 if t == 0 else hsT_view[:, :, t - 1, :])
                        # r gate (ready after m-tiles 0..7)
                        ar = gp.tile([128, 16], F32)
                        nc.vector.tensor_tensor(ar[:], ps[:, 0:16], xwt[:, 0:16], OP.add)
                        r_ = gp.tile([128, 16], F32)
                        nc.scalar.activation(r_[:], ar[:], AF.Sigmoid)
                        # n candidate (after n-block m-tiles) — runs under the
                        # z-block matmuls
                        hnb = gp.tile([128, 16], F32)
                        nc.vector.tensor_tensor(hnb[:], ps[:, 32:48], bhn_sb[:], OP.add)
                        rn = gp.tile([128, 16], F32)
                        nc.vector.tensor_tensor(rn[:], r_[:], hnb[:], OP.mult)
                        an = gp.tile([128, 16], F32)
                        nc.vector.tensor_tensor(an[:], rn[:],
                                                xwt[:, noff:noff + 16], OP.add)
                        n_ = gp.tile([128, 16], F32)
                        nc.scalar.activation(n_[:], an[:], AF.Tanh)
                        d = gp.tile([128, 16], F32)
                        nc.vector.tensor_tensor(
                            d[:].rearrange(kb_view, k=KH), hprev,
                            n_[:].rearrange(kb_view, k=KH), OP.subtract)
                        # z gate (after z-block m-tiles) — exposed tail
                        z_ = gp.tile([128, 16], F32)
                        if ZID:
                            # xw_z already accumulated into PSUM
                            nc.scalar.activation(z_[:], ps[:, 16:32], AF.Sigmoid)
                        else:
                            az = gp.tile([128, 16], F32)
                            nc.vector.tensor_tensor(az[:], ps[:, 16:32],
                                                    xwt[:, 16:32], OP.add)
                            nc.scalar.activation(z_[:], az[:], AF.Sigmoid)
                        zd = gp.tile([128, 16], F32)
                        nc.vector.tensor_tensor(zd[:], z_[:], d[:], OP.mult)
                        # h = n + z*(hprev - n), written straight to bf16 hsT
                        nc.vector.tensor_tensor(
                            hsT_view[:, :, t, :],
                            n_[:].rearrange(kb_view, k=KH),
                            zd[:].rearrange(kb_view, k=KH), OP.add)

                        if proj is not None:
                            emit_proj_out(*proj)

                    # ---------- projection tail (token block 3) ------------
                    tail = (([3] if interleave else [0, 1, 2, 3]) if proj else [])
                    for n in range(NV):
                        wt = load_wo_chunk(n)
                        for tb in tail:
                            emit_proj_out(emit_proj_mms(tb, n, wt), tb, n)
                if rep != reps - 1:
                    tc.strict_bb_all_engine_barrier()

    nc.compile()
    return nc


# ---------------------------------------------------------------------------
# host side
# ---------------------------------------------------------------------------

def _pack_colmajor(vec_2d, bpc_rows):
    """[bpc, H] f32 -> [128, 2*KH] with col = 2*k + b."""
    o = np.zeros((128, 2 * KH), np.float32)
    for k in range(KH):
        for b in range(bpc_rows):
            o[:, 2 * k + b] = vec_2d[b, k * 128:(k + 1) * 128]
    return o


def make_in_maps(encoder_state, targets, embed_table, Wi, Wh, bi, bhn, Wo, bo):
    encoder_state = np.asarray(encoder_state, np.float32)
    targets = np.asarray(targets)
    embed_table = np.asarray(embed_table, np.float32)
    Wi = np.asarray(Wi, np.float32)
    Wh = np.asarray(Wh, np.float32)
    bi = np.asarray(bi, np.float32)
    bhn = np.asarray(bhn, np.float32)
    Wo = np.asarray(Wo, np.float32)
    bo = np.asarray(bo, np.float32)

    import ml_dtypes
    emb_bf = embed_table.astype(ml_dtypes.bfloat16)
    wh_bf = Wh.astype(ml_dtypes.float8_e4m3 if WH_FP8 else ml_dtypes.bfloat16)
    wiA = np.zeros((EA, 3 * H), np.float32)
    wiA[:E] = Wi
    wiA[E] = bi
    wiA_bf = wiA.astype(ml_dtypes.bfloat16)
    wo_bf = Wo.astype(ml_dtypes.bfloat16)
    bo_row = bo.reshape(1, V).astype(ml_dtypes.bfloat16)

    bhn_pack = _pack_colmajor(np.broadcast_to(bhn, (BPC, H)), BPC)

    in_maps = []
    for c in range(NCORES):
        rows = slice(c * BPC, (c + 1) * BPC)
        # time-major token order: tok = t*2 + b
        tgt = np.ascontiguousarray(
            targets[rows].T).reshape(TOK, 1).astype(np.int32)
        h0 = _pack_colmajor(encoder_state[rows], BPC)
        in_maps.append({
            "tgt": tgt,
            "h0T": h0,
            "emb": emb_bf,
            "wh": wh_bf,
            "wiA": wiA_bf,
            "wo": wo_bf,
            "bo_in": bo_row,
            "bhnT": bhn_pack,
        })
    return in_maps


_NC_CACHE = {}


def get_nc(T_steps=T, reps=1):
    key = (T_steps, reps)
    if key not in _NC_CACHE:
        _NC_CACHE[key] = build(T_steps, reps)
    return _NC_CACHE[key]


def kernel(encoder_state, targets, embed_table, Wi, Wh, bi, bhn, Wo, bo):
    from concourse.bass_utils import run_bass_kernel_spmd
    nc = get_nc()
    in_maps = make_in_maps(encoder_state, targets, embed_table, Wi, Wh, bi,
                           bhn, Wo, bo)
    res = run_bass_kernel_spmd(nc, in_maps, list(range(NCORES)))
    outs = []
    for c in range(NCORES):
        o = res.results[c]["out"].astype(np.float32)
        # rows are tok = t*2 + b -> [BPC, T, V]
        outs.append(o.reshape(T, BPC, V).transpose(1, 0, 2))
    return np.concatenate(outs, axis=0)

